# revision 2
# baseline (speedup 1.0000x reference)
"""Self-contained Trainium2 Bass kernel for the CNN-LSTM-CTC model.

kernel(**inputs) takes the FULL (unsharded) numpy inputs of
reference.setup_inputs() and returns the full [128, 1600, 5] float32
output.  Internally: pure data parallel over 8 NeuronCores (batch 16
per core); each core runs a single fused Bass/Tile program:
conv stack (phase-decomposed GEMMs) -> 5 LSTM layers (feature-major
recurrence, 36 LDW+MM pairs per step, xg injected into PSUM via
identity matmuls, next layer's input GEMM interleaved into the PE idle
windows at demoted scheduler priority) -> dense CTC head.
fp16 matmul operands / fp32 accumulate and state.
"""
from contextlib import ExitStack
import time

import numpy as np
import jax
from jax.sharding import Mesh, PartitionSpec, NamedSharding

import bass_rust
import concourse.bass as bass
import concourse.tile as tile
import concourse.mybir as mybir
import concourse.bass2jax as b2j

# ======================================================================
# geometry / host-side prep
# ======================================================================
B = 16          # per-core batch
T = 8000
U = 1600        # output timesteps
J = 1605        # tau blocks (tau = 5j + p)
NJ = J * B      # conv moving columns, col = j*16 + b
NU = U * B      # lstm moving columns, col = u*16 + b
H = 384
G4 = 1536

# r-permutation for c2out partitions: position of r in the partition order
R_ORDER = [3, 4, 0, 1, 2]            # partition block -> r
R_POS = {r: i for i, r in enumerate(R_ORDER)}   # r -> partition block
# conv3 q' groups: (q', [dk3 list], partition range)
Q_GROUPS = []
for q in range(5):
    dks = [dk for dk in range(19) if (dk + 3) // 5 == q]
    rows = sorted(R_POS[(dk + 3) % 5] for dk in dks)
    lo, hi = rows[0] * 16, (rows[-1] + 1) * 16
    Q_GROUPS.append((q, dks, lo, hi))
# -> q0: rows [0:32), q1-3: [0:80), q4: [32:64)


import ml_dtypes

# Wh quantization: scale into the fp8 normal range; the inverse scale is
# folded into the gate activations (scale=) and the xg GEMM bias-add.
WH_SCALE = 48.0
WH_NP_DT = ml_dtypes.float8_e3m4


def f8(a):
    return np.ascontiguousarray(np.asarray(a, dtype=WH_NP_DT))


def f16(a):
    return np.ascontiguousarray(a, dtype=np.float16)


def f32(a):
    return np.ascontiguousarray(a, dtype=np.float32)


def prep_weights(inputs):
    """Prepare all weight/bias tensors (shared across cores)."""
    w1, b1 = np.asarray(inputs['conv1_w']), np.asarray(inputs['conv1_b'])
    w2, b2 = np.asarray(inputs['conv2_w']), np.asarray(inputs['conv2_b'])
    w3, b3 = np.asarray(inputs['conv3_w']), np.asarray(inputs['conv3_b'])
    Wi, Wh, bl = (np.asarray(inputs['lstm_Wi']), np.asarray(inputs['lstm_Wh']),
                  np.asarray(inputs['lstm_b']))
    dw, db = np.asarray(inputs['dense_w']), np.asarray(inputs['dense_b'])

    out = {}
    # conv1: lhsT1 [9, 20]; M order (p, co): idx = p*4+co
    lhsT1 = np.zeros((9, 20), np.float32)
    for o in range(9):
        for p in range(5):
            dk = o - p
            if 0 <= dk < 5:
                lhsT1[o, p * 4:(p + 1) * 4] = w1[dk, 0, :]
    out['lhsT1'] = f16(lhsT1)
    out['b1c'] = f32(np.tile(b1, 5))           # [20] bias per (p,co)

    # conv2: lhsT2 [36, 80]; K order (r, ci): idx = r*4+ci;
    # M order (R_POS[p], co): idx = R_POS[p]*16+co
    lhsT2 = np.zeros((36, 80), np.float32)
    for r in range(9):
        for ci in range(4):
            for p in range(5):
                dk = r - p
                if 0 <= dk < 5:
                    lhsT2[r * 4 + ci, R_POS[p] * 16:(R_POS[p] + 1) * 16] = \
                        w2[dk, ci, :]
    out['lhsT2'] = f16(lhsT2)
    b2c = np.zeros(80, np.float32)
    for p in range(5):
        b2c[R_POS[p] * 16:(R_POS[p] + 1) * 16] = b2
    out['b2c'] = f32(b2c)

    # conv3: lhsT3 [80, 5, 384]: [(rpos,ci), q', col] = w3[dk3, ci, col]
    lhsT3 = np.zeros((80, 5, 384), np.float32)
    for q, dks, lo, hi in Q_GROUPS:
        for dk in dks:
            rpos = R_POS[(dk + 3) % 5]
            for ci in range(16):
                lhsT3[rpos * 16 + ci, q, :] = w3[dk, ci, :]
    out['lhsT3'] = f16(lhsT3)
    out['b3c'] = f32(b3.reshape(3, 128))

    # lstm: WiT/WhT [5, 3, 12, 128(r), 128(c)] (compute layout)
    out['WiT'] = f16(Wi.reshape(5, 3, 128, 12, 128).transpose(0, 1, 3, 2, 4))
    out['WhT'] = f16(Wh.reshape(5, 3, 128, 12, 128).transpose(0, 1, 3, 2, 4))
    # device layouts [5, 128(r), 3, 12, 128(c)] so the DMA is contiguous.
    # Wh is loaded as fp8e4m3 stationary operands: FWL reads 4 fp8/cycle
    # (vs 2 fp16), halving the per-step LDWEIGHTS bottleneck.
    out['WiT_dev'] = f16(out['WiT'].transpose(0, 3, 1, 2, 4))
    out['WhT_dev'] = f8(out['WhT'].astype(np.float32).transpose(0, 3, 1, 2, 4)
                        * WH_SCALE)
    out['bLc'] = f32(bl.reshape(5, 12, 128))

    # dense: dWT [3, 128, 5]; device layout [128, 3, 5]
    out['dWT'] = f16(dw.reshape(3, 128, 5))
    out['dWT_dev'] = f16(dw.reshape(3, 128, 5).transpose(1, 0, 2))
    out['dbc'] = f32(db)                       # [5]
    return out


def prep_x(x_core):
    """x_core: [B, T] float32 -> rhs1 [9, NJ] float16.
    rhs1[o, j*16+b] = x[b, 5j + o - 12]  (0 outside [0,T))."""
    xb = np.asarray(x_core, dtype=np.float32)
    assert xb.shape == (B, T)
    rhs1 = np.zeros((9, J, B), np.float32)
    for o in range(9):
        # t index = 5j + o - 12 for j in [0, J)
        tidx = 5 * np.arange(J) + o - 12
        valid = (tidx >= 0) & (tidx < T)
        rhs1[o, valid, :] = xb[:, tidx[valid]].T
    return f16(rhs1.reshape(9, NJ))


ZERO_J = [0, 1, 1602, 1603, 1604]   # tau blocks to zero in c1out/c2out


# ======================================================================
# BIR post-processing (walrus supports only 1 sem wait per instruction)
# ======================================================================
MAX_WAITS = 1


def split_excess_waits(nc, max_waits=MAX_WAITS):
    """Walrus codegen only supports `max_waits` semaphore waits per
    instruction; tile's tail drain (and occasionally other instructions)
    can carry more. Hoist the excess into preceding same-engine NoOps."""
    n_fixed = 0
    for fn in nc.m.functions:
        for b in fn.blocks:
            insts = b.instructions
            out = []
            dirty = False
            for inst in insts:
                si = inst.sync_info
                if si is not None and len(si.on_wait) > max_waits:
                    waits = list(si.on_wait)
                    extra, keep = waits[:-max_waits], waits[-max_waits:]
                    k = 0
                    while extra:
                        chunk, extra = extra[:max_waits], extra[max_waits:]
                        nop = mybir.InstNoOp(
                            name=f"{inst.name}_waitsplit{k}", ins=[], outs=[])
                        nop.engine = inst.engine
                        nop.sync_info = bass_rust.SyncInfo(
                            on_wait=chunk, on_update=[])
                        out.append(nop)
                        k += 1
                    si.on_wait = keep
                    n_fixed += 1
                    dirty = True
                out.append(inst)
            if dirty:
                b.instructions = out
    return n_fixed


# ======================================================================
# bass/tile program builder
# ======================================================================
DT16 = mybir.dt.float16
DT32 = mybir.dt.float32
DT8 = mybir.dt.float8e3
AF = mybir.ActivationFunctionType
ALU = mybir.AluOpType


def perm(ap, order):
    """Permute the dims of an AP, dropping unit dims (DRAM-side APs only)."""
    dims = [ap.ap[i] for i in order]
    dims = [d for d in dims if d[1] != 1] or dims[:1]
    return bass.AP(tensor=ap.tensor, offset=ap.offset, ap=dims)


class GemmEmitter:
    """Emits the xgb GEMM for layer `l` column-range by column-range."""

    def __init__(self, tc, nc, l, xin, xgb, WiT_d, bL_d):
        self.tc, self.nc, self.l = tc, nc, l
        self.xin, self.xgb = xin, xgb
        self.ctx = ExitStack()
        self.wp = self.ctx.enter_context(tc.tile_pool(name=f"gw{l}", bufs=1))
        self.io = self.ctx.enter_context(tc.tile_pool(name=f"gio{l}", bufs=3))
        self.ps = self.ctx.enter_context(
            tc.tile_pool(name=f"gps{l}", bufs=2, space="PSUM"))
        self.WiT_s = self.wp.tile([128, 3, 12, 128], DT16, tag=f"giw{l}")
        nc.sync.dma_start(self.WiT_s[:], WiT_d[l])
        self.bL_s = self.wp.tile([128, 12], DT32, tag=f"gib{l}")
        nc.sync.dma_start(self.bL_s[:], bL_d[:, l, :])

    def emit_cols(self, u0, nu_total, rev=False):
        nc = self.nc
        starts = list(range(u0, u0 + nu_total, 32))
        if rev:
            starts.reverse()
        for uo in starts:
            nu = min(32, u0 + nu_total - uo)
            n = nu * B
            rx = self.io.tile([128, 3, 512], DT16, tag=f"grx{self.l}")
            nc.sync.dma_start(rx[:, :, :n],
                              perm(self.xin[:, :, uo:uo + nu, :], (1, 0, 2, 3)))
            self._chunk(lambda k: rx[:, k, :n], uo, nu)

    def emit_ring(self, ring, t_lo, ublock):
        # read the producing recurrence's SBUF ring directly (slot = t - t_lo)
        cs = min(16, ublock)
        for uo in range(t_lo, t_lo + ublock, cs):
            s0 = uo - t_lo
            self._chunk(lambda k: ring[:, s0:s0 + cs, k, :], uo, cs)

    def _chunk(self, rhs_k, uo, nu):
        nc = self.nc
        n = nu * B
        for m in range(12):
            pg = self.ps.tile([128, 512], DT32, tag=f"gpg{self.l}")
            for k in range(3):
                nc.tensor.matmul(pg[:, :n], self.WiT_s[:, k, m, :],
                                 rhs_k(k), start=(k == 0), stop=(k == 2))
            og = self.io.tile([128, 512], DT16, tag=f"gog{self.l}")
            nc.scalar.activation(og[:, :n], pg[:, :n], AF.Identity,
                                 bias=self.bL_s[:, m:m + 1], scale=WH_SCALE)
            nc.sync.dma_start(self.xgb[:, m, uo:uo + nu, :], og[:, :n])

    def close(self):
        self.ctx.close()


class DenseEmitter:
    """Emits the dense CTC head column-range by column-range."""

    def __init__(self, tc, nc, xin, dWT_d, db_d, y_d):
        self.tc, self.nc = tc, nc
        self.xin, self.y_d = xin, y_d
        self.ctx = ExitStack()
        self.wp = self.ctx.enter_context(tc.tile_pool(name="dw", bufs=1))
        self.io = self.ctx.enter_context(tc.tile_pool(name="dio", bufs=3))
        self.ps = self.ctx.enter_context(
            tc.tile_pool(name="dps", bufs=2, space="PSUM"))
        self.dW_s = self.wp.tile([128, 3, 5], DT16, tag="dwt")
        nc.sync.dma_start(self.dW_s[:], dWT_d[:])
        self.db_s = self.wp.tile([5, 1], DT32, tag="dbt")
        nc.sync.dma_start(self.db_s[:], db_d[:])

    def emit_cols(self, u0, nu_total):
        nc = self.nc
        for uo in range(u0, u0 + nu_total, 32):
            nu = min(32, u0 + nu_total - uo)
            n = nu * B
            rx = self.io.tile([128, 3, 512], DT16, tag="drx")
            nc.sync.dma_start(rx[:, :, :n],
                              perm(self.xin[:, :, uo:uo + nu, :], (1, 0, 2, 3)))
            self._chunk(lambda k: rx[:, k, :n], uo, nu)

    def emit_ring(self, ring, t_lo, ublock):
        cs = min(32, ublock)
        for uo in range(t_lo, t_lo + ublock, cs):
            s0 = uo - t_lo
            self._chunk(lambda k: ring[:, s0:s0 + cs, k, :], uo, cs)

    def _chunk(self, rhs_k, uo, nu):
        nc = self.nc
        n = nu * B
        pd = self.ps.tile([5, 512], DT32, tag="pd")
        for k in range(3):
            nc.tensor.matmul(pd[:, :n], self.dW_s[:, k, :], rhs_k(k),
                             start=(k == 0), stop=(k == 2))
        oy = self.io.tile([5, 512], DT32, tag="oy")
        nc.scalar.activation(oy[:, :n], pd[:, :n], AF.Identity,
                             bias=self.db_s[:])
        nc.sync.dma_start(self.y_d[:, uo:uo + nu, :], oy[:, :n])

    def close(self):
        self.ctx.close()


def build(u_steps=1600, layers=5, with_conv=True, ublock=64, cb_j=30):
    assert u_steps % ublock == 0
    NU = u_steps * B
    nc = bass.Bass("TRN2", target_bir_lowering=False, debug=False)

    def din(name, shape, dt=DT16):
        return nc.dram_tensor(name, shape, dt, kind="ExternalInput").ap()

    def scratch(name, shape, dt=DT16):
        return nc.dram_tensor(name, shape, dt, kind="Internal").ap()

    if with_conv:
        rhs1_d = din("rhs1", [9, NJ])
        lhsT1_d = din("lhsT1", [9, 20])
        b1_d = din("b1c", [20, 1], DT32)
        lhsT2_d = din("lhsT2", [36, 80])
        b2_d = din("b2c", [80, 1], DT32)
        lhsT3_d = din("lhsT3", [80, 5, 384])
        b3_d = din("b3c", [128, 3], DT32)
    else:
        xt0_d = din("xt0", [3, 128, u_steps, B])
    if layers:
        ident_d = din("ident", [128, 128], DT8)
        WiT_d = din("WiT", [layers, 128, 3, 12, 128])
        WhT_d = din("WhT", [layers, 128, 3, 12, 128], DT8)
        bL_d = din("bLc", [128, layers, 12], DT32)
    dWT_d = din("dWT", [128, 3, 5])
    db_d = din("dbc", [5, 1], DT32)
    y_d = nc.dram_tensor("y", [5, u_steps, B], DT32, kind="ExternalOutput").ap()

    c1_hbm = scratch("c1s", [20, J + 2, B]) if with_conv else None
    c2_hbm = scratch("c2s", [80, J, B]) if with_conv else None
    xgb = [scratch("xgbs0", [128, 12, u_steps, B]),
           scratch("xgbs1", [128, 12, u_steps, B])]
    xta_hbm = scratch("xtas", [3, 128, u_steps, B]) if with_conv else xt0_d

    with tile.TileContext(nc) as tc:
        if with_conv:
            conv_stage(tc, nc, u_steps, cb_j, rhs1_d, lhsT1_d, b1_d, lhsT2_d,
                       b2_d, lhsT3_d, b3_d, c1_hbm, c2_hbm, xta_hbm)

        if layers:
            g0 = GemmEmitter(tc, nc, 0, xta_hbm, xgb[0], WiT_d, bL_d)
            with tc.high_priority(offset=-100_000_000):
                g0.emit_cols(0, u_steps, rev=True)  # layer 0 consumes t desc.
            g0.close()
        for l in range(layers):
            if l + 1 < layers:
                nxt = GemmEmitter(tc, nc, l + 1, None, xgb[(l + 1) % 2],
                                  WiT_d, bL_d)
            else:
                nxt = DenseEmitter(tc, nc, None, dWT_d, db_d, y_d)
            recurrence(tc, nc, l, u_steps, ublock, xgb[l % 2],
                       WhT_d, ident_d, reverse=(l % 2 == 0), worker=nxt)
            nxt.close()
        if not layers:
            d = DenseEmitter(tc, nc, xta_hbm, dWT_d, db_d, y_d)
            d.emit_cols(0, u_steps)
            d.close()

    return nc


def conv_stage(tc, nc, u_steps, cb_j, rhs1_d, lhsT1_d, b1_d, lhsT2_d,
               b2_d, lhsT3_d, b3_d, c1_hbm, c2_hbm, xta_hbm):
    ctx = ExitStack()
    wp = ctx.enter_context(tc.tile_pool(name="cw", bufs=1))
    io = ctx.enter_context(tc.tile_pool(name="cio", bufs=3))
    ps = ctx.enter_context(tc.tile_pool(name="cps", bufs=2, space="PSUM"))

    lhsT1_s = wp.tile([9, 20], DT16)
    nc.sync.dma_start(lhsT1_s[:], lhsT1_d[:])
    b1_s = wp.tile([20, 1], DT32)
    nc.sync.dma_start(b1_s[:], b1_d[:])
    lhsT2_s = wp.tile([36, 80], DT16)
    nc.sync.dma_start(lhsT2_s[:], lhsT2_d[:])
    b2_s = wp.tile([80, 1], DT32)
    nc.sync.dma_start(b2_s[:], b2_d[:])
    lhsT3_s = wp.tile([80, 5, 384], DT16)
    nc.sync.dma_start(lhsT3_s[:], lhsT3_d[:])
    b3_s = wp.tile([128, 3], DT32)
    nc.sync.dma_start(b3_s[:], b3_d[:])
    zeros_s = wp.tile([80, 80], DT16)
    nc.vector.memset(zeros_s[:], 0)

    CB = cb_j * B
    jblocks = [(j0, min(cb_j, J - j0)) for j0 in range(0, J, cb_j)]
    for j0, nj in jblocks:
        n = nj * B
        r1 = io.tile([9, CB], DT16, tag="r1")
        nc.sync.dma_start(r1[:, :n], rhs1_d[:, j0 * B:j0 * B + n])
        p1 = ps.tile([20, CB], DT32, tag="p1")
        nc.tensor.matmul(p1[:, :n], lhsT1_s[:], r1[:, :n], start=True, stop=True)
        sg1 = io.tile([20, CB], DT32, tag="sg1")
        nc.scalar.activation(sg1[:, :n], p1[:, :n], AF.Sigmoid, bias=b1_s[:])
        o1 = io.tile([20, CB], DT16, tag="o1")
        nc.vector.scalar_tensor_tensor(o1[:, :n], p1[:, :n], b1_s[:],
                                       sg1[:, :n], op0=ALU.add, op1=ALU.mult)
        nc.sync.dma_start(c1_hbm[:, j0 + 1:j0 + 1 + nj, :], o1[:, :n])
    nc.sync.dma_start(c1_hbm[:, 0:3, :], zeros_s[:20, :48])
    nc.sync.dma_start(c1_hbm[:, J - 2:J + 2, :], zeros_s[:20, :64])

    for j0, nj in jblocks:
        n = nj * B
        r2 = io.tile([36, CB], DT16, tag="r2")
        nc.sync.dma_start(r2[0:8, :n], c1_hbm[12:20, j0:j0 + nj, :])
        nc.sync.dma_start(r2[8:28, :n], c1_hbm[0:20, j0 + 1:j0 + 1 + nj, :])
        nc.sync.dma_start(r2[28:36, :n], c1_hbm[0:8, j0 + 2:j0 + 2 + nj, :])
        p2 = ps.tile([80, CB], DT32, tag="p2")
        nc.tensor.matmul(p2[:, :n], lhsT2_s[:], r2[:, :n], start=True, stop=True)
        sg2 = io.tile([80, CB], DT32, tag="sg2")
        nc.scalar.activation(sg2[:, :n], p2[:, :n], AF.Sigmoid, bias=b2_s[:])
        o2 = io.tile([80, CB], DT16, tag="o2")
        nc.vector.scalar_tensor_tensor(o2[:, :n], p2[:, :n], b2_s[:],
                                       sg2[:, :n], op0=ALU.add, op1=ALU.mult)
        nc.sync.dma_start(c2_hbm[:, j0:j0 + nj, :], o2[:, :n])
    nc.sync.dma_start(c2_hbm[:, 0:2, :], zeros_s[:, :32])
    nc.sync.dma_start(c2_hbm[:, J - 3:J, :], zeros_s[:, :48])

    UC = 32
    for u0 in range(0, u_steps, UC):
        nu = min(UC, u_steps - u0)
        r3 = io.tile([80, UC + 4, B], DT16, tag="r3")
        nc.sync.dma_start(r3[:, :nu + 4, :], c2_hbm[:, u0:u0 + nu + 4, :])
        for m in range(3):
            p3 = ps.tile([128, UC * B], DT32, tag="p3")
            for q, _dks, lo, hi in Q_GROUPS:
                nc.tensor.matmul(
                    p3[:, :nu * B],
                    lhsT3_s[lo:hi, q, m * 128:(m + 1) * 128],
                    r3[lo:hi, q:q + nu, :],
                    start=(q == 0), stop=(q == 4))
            sg3 = io.tile([128, UC * B], DT32, tag="sg3")
            nc.scalar.activation(sg3[:, :nu * B], p3[:, :nu * B], AF.Sigmoid,
                                 bias=b3_s[:, m:m + 1])
            o3 = io.tile([128, UC * B], DT16, tag="o3")
            nc.vector.scalar_tensor_tensor(o3[:, :nu * B], p3[:, :nu * B],
                                           b3_s[:, m:m + 1], sg3[:, :nu * B],
                                           op0=ALU.add, op1=ALU.mult)
            nc.sync.dma_start(xta_hbm[m, :, u0:u0 + nu, :], o3[:, :nu * B])
    ctx.close()


def recurrence(tc, nc, l, u_steps, ublock, xgb_hbm,
               WhT_d, ident_d, reverse, worker=None):
    c2 = ExitStack()
    wp = c2.enter_context(tc.tile_pool(name=f"rw{l}", bufs=1))
    xp = c2.enter_context(tc.tile_pool(name=f"rxg{l}", bufs=2))
    ep = c2.enter_context(tc.tile_pool(name=f"rep{l}", bufs=2))
    zp = c2.enter_context(tc.tile_pool(name=f"rz{l}", bufs=2, space="PSUM"))
    zp2 = c2.enter_context(tc.tile_pool(name=f"rzg{l}", bufs=2, space="PSUM"))
    zp3 = c2.enter_context(tc.tile_pool(name=f"rzo{l}", bufs=2, space="PSUM"))
    WhT_s = wp.tile([128, 3, 12, 128], DT8)
    nc.sync.dma_start(WhT_s[:], WhT_d[l])
    ident_s = wp.tile([128, 128], DT8)
    nc.sync.dma_start(ident_s[:], ident_d[:])
    c_s = wp.tile([128, 3, B], DT32)
    nc.vector.memset(c_s[:], 0)
    hzero = wp.tile([128, 3, B], DT16)
    nc.vector.memset(hzero[:], 0)
    ring0 = wp.tile([128, ublock, 3, B], DT16, tag="ring0")
    ring1 = wp.tile([128, ublock, 3, B], DT16, tag="ring1")
    rings = [ring0, ring1]

    nblocks = u_steps // ublock
    for blk in range(nblocks):
        par = blk % 2
        ring = rings[par]
        t_lo = (u_steps - (blk + 1) * ublock) if reverse else blk * ublock
        xg_s = xp.tile([128, 12, ublock, B], DT16, tag="xg")
        nc.sync.dma_start(xg_s[:], xgb_hbm[:, :, t_lo:t_lo + ublock, :])
        for s in range(ublock):
            step = blk * ublock + s
            slot = (ublock - 1 - s) if reverse else s
            if step == 0:
                hp = hzero
            elif s == 0:
                pslot = 0 if reverse else ublock - 1
                hp = rings[1 - par][:, pslot, :, :]
            else:
                hp = ring[:, slot + (1 if reverse else -1), :, :]
            # Gate matmuls split (i,f | g | o) into separate psum banks so
            # each group's post-processing overlaps later groups' matmuls;
            # xg is injected by identity matmuls during the h-wait window.
            pzif = zp.tile([128, 6, B], DT32, tag="pzif")
            pzg = zp2.tile([128, 3, B], DT32, tag="pzg")
            pzo = zp3.tile([128, 3, B], DT32, tag="pzo")
            nc.tensor.matmul(pzif[:], ident_s[:], xg_s[:, 0:6, slot, :],
                             start=True, stop=False, skip_group_check=True)
            nc.tensor.matmul(pzg[:], ident_s[:], xg_s[:, 6:9, slot, :],
                             start=True, stop=False, skip_group_check=True)
            nc.tensor.matmul(pzo[:], ident_s[:], xg_s[:, 9:12, slot, :],
                             start=True, stop=False, skip_group_check=True)
            seq = [(pzif, 0, 0), (pzif, 1, 1), (pzg, 0, 6),
                   (pzif, 2, 2), (pzif, 3, 3), (pzg, 1, 7),
                   (pzif, 4, 4), (pzif, 5, 5), (pzg, 2, 8),
                   (pzo, 0, 9), (pzo, 1, 10), (pzo, 2, 11)]
            for dst, lm, gm in seq:
                for k in range(3):
                    nc.tensor.matmul(dst[:, lm, :], WhT_s[:, k, gm, :],
                                     hp[:, k, :], start=False,
                                     stop=(k == 2), skip_group_check=True)
            sif = ep.tile([128, 6, B], DT32, tag="sif")
            nc.scalar.activation(sif[:], pzif[:], AF.Sigmoid,
                                 scale=1.0 / WH_SCALE)
            nc.vector.tensor_mul(c_s[:], sif[:, 3:6, :], c_s[:])
            tg = ep.tile([128, 3, B], DT32, tag="tg")
            nc.scalar.activation(tg[:], pzg[:], AF.Tanh,
                                 scale=1.0 / WH_SCALE)
            ig = ep.tile([128, 3, B], DT32, tag="ig")
            nc.vector.tensor_mul(ig[:], sif[:, 0:3, :], tg[:])
            nc.vector.tensor_add(c_s[:], c_s[:], ig[:])
            tc_ = ep.tile([128, 3, B], DT32, tag="tc")
            nc.scalar.activation(tc_[:], c_s[:], AF.Tanh)
            so = ep.tile([128, 3, B], DT32, tag="so")
            nc.scalar.activation(so[:], pzo[:], AF.Sigmoid,
                                 scale=1.0 / WH_SCALE)
            nc.vector.tensor_mul(ring[:, slot, :, :], so[:], tc_[:])
        if worker is not None:
            with tc.high_priority(offset=-100_000_000):
                worker.emit_ring(ring, t_lo, ublock)
    c2.close()


# ======================================================================
# PJRT SPMD runner
# ======================================================================
P = PartitionSpec


class SpmdRunner:
    def __init__(self, nc, n_cores=8):
        b2j.install_neuronx_cc_hook()
        self.nc = nc
        partition_name = (nc.partition_id_tensor.name
                          if nc.partition_id_tensor else None)
        self.n_cores = n_cores
        in_names, out_names, out_avals, zero_outs = [], [], [], []
        for alloc in nc.m.functions[0].allocations:
            if not isinstance(alloc, mybir.MemoryLocationSet):
                continue
            name = alloc.memorylocations[0].name
            if alloc.kind == "ExternalInput":
                if name != partition_name:
                    in_names.append(name)
            elif alloc.kind == "ExternalOutput":
                shape = tuple(alloc.tensor_shape)
                dtype = mybir.dt.np(alloc.dtype)
                out_names.append(name)
                out_avals.append(jax.core.ShapedArray(shape, dtype))
                zero_outs.append(np.zeros(shape, dtype))
        self.in_names, self.out_names = in_names, out_names
        self.out_avals, self.zero_outs = out_avals, zero_outs
        n_params = len(in_names)
        all_names = in_names + out_names
        if partition_name is not None:
            all_names = all_names + [partition_name]
        all_names = tuple(all_names)

        def _body(*args):
            operands = list(args)
            if partition_name is not None:
                operands.append(b2j.partition_id_tensor())
            outs = b2j._bass_exec_p.bind(
                *operands,
                out_avals=tuple(out_avals),
                in_names=all_names,
                out_names=tuple(out_names),
                lowering_input_output_aliases=(),
                sim_require_finite=True,
                sim_require_nnan=True,
                nc=nc,
            )
            return tuple(outs)

        devices = jax.devices()[:n_cores]
        self.mesh = Mesh(np.asarray(devices), ("core",))
        self.sharding = NamedSharding(self.mesh, P("core"))
        n_outs = len(out_names)
        self.fn = jax.jit(
            jax.shard_map(_body, mesh=self.mesh,
                          in_specs=(P("core"),) * (n_params + n_outs),
                          out_specs=(P("core"),) * n_outs,
                          check_vma=False),
            donate_argnums=tuple(range(n_params, n_params + n_outs)),
            keep_unused=True,
        )

    def place_inputs(self, in_maps):
        concat = [np.concatenate([np.asarray(m[n]) for m in in_maps], axis=0)
                  for n in self.in_names]
        return [jax.device_put(a, self.sharding) for a in concat]

    def _zeros(self):
        return [jax.device_put(
            np.zeros((self.n_cores * z.shape[0], *z.shape[1:]), z.dtype),
            self.sharding) for z in self.zero_outs]

    def run(self, in_dev, time_reps=0):
        """Returns (per-core outputs list, exec_seconds or None)."""
        out = self.fn(*in_dev, *self._zeros())
        jax.block_until_ready(out)
        best = None
        for _ in range(time_reps):
            zs = self._zeros()
            jax.block_until_ready(zs)
            jax.block_until_ready(in_dev)
            t0 = time.perf_counter()
            out = self.fn(*in_dev, *zs)
            jax.block_until_ready(out)
            dt = time.perf_counter() - t0
            best = dt if best is None else min(best, dt)
        results = []
        for c in range(self.n_cores):
            results.append({
                n: np.asarray(out[i]).reshape(self.n_cores, *self.out_avals[i].shape)[c]
                for i, n in enumerate(self.out_names)})
        return results, best


# ======================================================================
# kernel entry
# ======================================================================
_CACHE = {}


def kernel(**inputs):
    x = np.asarray(inputs['x'], dtype=np.float32)   # [128, 8000, 1]
    n_cores = 8
    W = prep_weights(inputs)

    shared = {
        'lhsT1': W['lhsT1'], 'b1c': W['b1c'].reshape(20, 1),
        'lhsT2': W['lhsT2'], 'b2c': W['b2c'].reshape(80, 1),
        'lhsT3': W['lhsT3'], 'b3c': np.ascontiguousarray(W['b3c'].T),
        'WiT': W['WiT_dev'], 'WhT': W['WhT_dev'],
        'bLc': np.ascontiguousarray(W['bLc'].transpose(2, 0, 1)) * WH_SCALE,
        'dWT': W['dWT_dev'], 'dbc': W['dbc'].reshape(5, 1),
        'ident': f8(np.eye(128, dtype=np.float32)),
    }
    in_maps = []
    for c in range(n_cores):
        m = dict(shared)
        m['rhs1'] = prep_x(x[c * B:(c + 1) * B, :, 0])
        in_maps.append(m)

    if 'runner' not in _CACHE:
        nc = build(u_steps=1600, layers=5, with_conv=True, ublock=64)
        split_excess_waits(nc)
        _CACHE['runner'] = SpmdRunner(nc, n_cores)
    runner = _CACHE['runner']
    in_dev = runner.place_inputs(in_maps)
    results, best = runner.run(in_dev, time_reps=int(_CACHE.get('reps', 0)))
    _CACHE['last_time_s'] = best

    out = np.empty((128, 1600, 5), np.float32)
    for c in range(n_cores):
        out[c * B:(c + 1) * B] = results[c]['y'].transpose(2, 1, 0)
    return out

